# revision 30
# baseline (speedup 1.0000x reference)
"""Graphormer3D encoder layer on 8 Trainium2 NeuronCores.

Sharding: data-parallel over the 16 graphs (2 per core); params replicated.
On-chip layout is feature-major (x^T: [feature, token]) with fp32 PSUM
accumulation. Structure (~1.3x over a straightforward fp16 port):
  - QKV and out-proj matmuls run in fp8(e4m3) DoubleRow mode (K=256 per
    instruction, ~2x); weights are quantized UNSCALED (the qk softmax scale
    is applied at the PSUM->SBUF copy) to stay out of e4m3's subnormal range.
  - Attention computes scores TRANSPOSED ([k, q]) so no probs/V transposes
    are needed: V is projected token-major with a per-head ones-column that
    yields softmax row-sums for free in partition 96 of the attn PSUM; 1/sum
    is broadcast across partitions with a K=1 ones-matmul + fast-approx
    reciprocal; attn values are scaled x8 into fp8 (the /8 is folded into
    the mask row host-side). The attention loop is software-pipelined
    (scores(i) | attn(i-1) | normalize(i-2)) so no PE instruction waits on
    same-iteration ScalarE/DVE results. Scores PSUM is paired [128,1024] so
    ScalarE exp runs 2 (not 4) ops per head (~330ns fixed cost per op).
    Softmax bias: k-tiles 0,1 add raw bias via identity-matmul on TensorE;
    k-tiles 2,3 multiply host-precomputed exp(bias) on GpSimd.
  - attn8/out-proj weights live in a head-PADDED 8x128 feature space (pad
    rows zero-weighted) so the fp8 store is one DVE op per head.
  - LN1 row stats (mean/rstd of the input) are host-precomputed and
    broadcast with K=1 ones-matmuls; LN2 reduces on TensorE via ones-matmul.
  - Phases 4/5 are pipelined by graph half: out(h0), LN2(h0), out(h1),
    fc1(h0), LN2(h1), fc2(h0), fc1(h1), fc2(h1) - LN2's serial stats chain
    hides under fc PE work. FFN stays fp16 (fp8 fails the 2e-2 gate).
  - PE warm-up via repeated idempotent mask-broadcast matmuls covers the
    ~13us DMA ramp and holds the HAM clock gate at full rate.
  - fc weights prefetch during attention; exp-bias streams 4 heads ahead;
    DMA dispatch is spread across the sync and gpsimd queues.
"""
import numpy as np

N_NODE, N_GRAPH, D = 512, 16, 768
H, HD, FFN = 8, 96, 3072
EPS = 1e-5
NC = 8            # cores
G = 2             # graphs per core
T = G * N_NODE    # tokens per core (1024)
KC = D // 128     # 6 feature chunks
C3 = D // 256     # 3 DoubleRow chunks
FC = FFN // 128   # 24 ffn chunks
NQT = N_NODE // 128  # 4 k tiles per graph
HLF = (slice(0, 512), slice(512, 1024))
QS = HD ** -0.5
ASCL = 8.0        # attn values scaled x8 before fp8; /8 folded into mask row
VROW = 784        # v token-major row: 2 groups x (4 heads x 97 + 4 pad)
VGRP = 392

_cached = {}


def _vbase(h):
    return (h // 4) * VGRP + (h % 4) * 97


def _head_segments(f0):
    """attn feature range [f0, f0+96) -> [(c, j, ki, length, hd0)] in the
    DoubleRow chunk layout (feature = 256c + 128j + ki)."""
    segs, f = [], f0
    while f < f0 + 96:
        c, rem = divmod(f, 256)
        j, ki = divmod(rem, 128)
        ln = min(128 - ki, f0 + 96 - f)
        segs.append((c, j, ki, ln, f - f0))
        f += ln
    return segs


def _allow(p):
    if p % 128 == 0:
        return 128
    if p % 64 == 0:
        return 64
    return 32


def _split_pp(src0, dst0, ln):
    """split a partition range so src/dst starts obey the HW block rule"""
    out = []
    while ln > 0:
        st = min(_allow(src0 % 128), _allow(dst0 % 128), ln)
        out.append((src0, dst0, st))
        src0 += st
        dst0 += st
        ln -= st
    return out


def _qk_segments(m):
    """chunk m of packed q/k psum rows -> [(r0, length, head, row_in_head)]"""
    f0 = 128 * (m % 6)
    segs, r = [], 0
    while r < 128:
        f = f0 + r
        h, a = divmod(f, 96)
        ln = min(96 - a, 128 - r)
        segs.append((r, ln, h, a))
        r += ln
    return segs


def _build():
    import concourse.bass as bass
    import concourse.mybir as mybir
    import concourse.tile as tile
    import concourse.bacc as bacc
    from contextlib import ExitStack

    F8 = mybir.dt.float8e4
    F16 = mybir.dt.float16
    F32 = mybir.dt.float32
    AF = mybir.ActivationFunctionType
    OP = mybir.AluOpType
    DR = mybir.MatmulPerfMode.DoubleRow

    nc = bacc.Bacc("TRN2", target_bir_lowering=False, debug=False, num_devices=NC)

    di = lambda name, shape, dt: nc.declare_dram_parameter(name, shape, dt, isOutput=False)
    xt_d = di("xt", [KC, 128, T], F16)
    bias_d = di("biasT", [G * H, 128, NQT, N_NODE], F16)
    mask_d = di("maskrow", [1, T], F16)
    wqk_d = di("wqk8", [C3, 128, 2, 2 * D], F8)
    wv_d = di("wv8", [C3, 128, 2, VROW], F8)
    wo_d = di("wo8", [4, 128, 2, D], F8)
    wfc1_d = di("wfc1", [KC, 128, FFN], F16)
    wfc2_d = di("wfc2", [FC, 128, D], F16)
    cb32_d = di("cb32", [128, 76], F32)
    lnrow_d = di("lnrow", [1, 2, T], F16)
    cb16_d = di("cb16", [128, 1040], F16)
    yt_d = nc.declare_dram_parameter("yt", [KC, 128, T], F16, isOutput=True)

    with tile.TileContext(nc) as tc, ExitStack() as top:
        const = top.enter_context(tc.tile_pool(name="const", bufs=1))

        cb16 = const.tile([128, 1040], F16, tag="cb16")
        nc.gpsimd.dma_start(cb16[:], cb16_d[:])
        mask_sb = const.tile([1, T], F16, tag="mask")
        nc.gpsimd.dma_start(mask_sb[:], mask_d[:])
        lnrow = const.tile([1, 2, T], F16, tag="lnrow")
        nc.gpsimd.dma_start(lnrow[:], lnrow_d[:])
        cb32 = const.tile([128, 76], F32, tag="cb32")
        nc.gpsimd.dma_start(cb32[:], cb32_d[:])
        ones = cb16[:, 0:128]
        ident = cb16[:, 128:256]
        bvb = cb16[:, 256:256 + VROW]
        bqk = cb32[0:96, 0:16]
        bout = cb32[:, 16:22]
        bfc1 = cb32[:, 22:46]
        bfc2 = cb32[:, 46:52]
        g1 = cb32[:, 52:58]
        b1 = cb32[:, 58:64]
        g2 = cb32[:, 64:70]
        b2 = cb32[:, 70:76]
        eps_sb = const.tile([128, 1], F32, tag="eps")
        nc.vector.memset(eps_sb[:], EPS)

        # long-lived pools (LIFO pool stack: order = reverse close order)
        mb_pool = top.enter_context(tc.tile_pool(name="maskb", bufs=1))
        y1_pool = top.enter_context(tc.tile_pool(name="y1", bufs=KC))
        yo_pool = top.enter_context(tc.tile_pool(name="yo", bufs=2))
        wfc1_pool = top.enter_context(tc.tile_pool(name="wfc1", bufs=KC))
        wf1_t = [wfc1_pool.tile([128, FFN], F16, tag="wfc1", name=f"wfc1_{i}") for i in range(KC)]

        s_x = ExitStack()
        s_wout = ExitStack()
        s_attn = ExitStack()
        s_qv = ExitStack()
        s_eb = ExitStack()
        s_w8 = ExitStack()

        x_pool = s_x.enter_context(tc.tile_pool(name="x", bufs=KC))
        # x DMA dispatched first on the sync queue
        x_tiles = []
        for k in range(KC):
            xt = x_pool.tile([128, T], F16, tag="x", name=f"x_{k}")
            (nc.sync if k % 2 == 0 else nc.scalar).dma_start(xt[:], xt_d[k])
            x_tiles.append(xt)
        wout_pool = s_wout.enter_context(tc.tile_pool(name="wout", bufs=4))
        wo_t = [wout_pool.tile([128, 2, D], F8, tag="wo", name=f"wo_{c}") for c in range(4)]
        attn8_pool = s_attn.enter_context(tc.tile_pool(name="attn8", bufs=4))
        attn8_t = [attn8_pool.tile([128, 2, T], F8, tag="attn8", name=f"attn8_{c}") for c in range(4)]
        for c in range(4):
            nc.vector.memset(attn8_t[c][96:128, :, :], 0.0)
        qkv_pool = s_qv.enter_context(tc.tile_pool(name="qkv", bufs=1))
        q_sb = qkv_pool.tile([HD, H * T], F16, tag="q")
        k_sb = qkv_pool.tile([HD, H * T], F16, tag="k")
        v_pool = s_qv.enter_context(tc.tile_pool(name="vtok", bufs=G * NQT))
        v_t = [v_pool.tile([128, VROW], F16, tag="v", name=f"v_{t}") for t in range(G * NQT)]
        eb_pool = s_eb.enter_context(tc.tile_pool(name="ebias", bufs=4))
        wqk_pool = s_w8.enter_context(tc.tile_pool(name="wqk", bufs=C3))
        wv_pool = s_w8.enter_context(tc.tile_pool(name="wv", bufs=C3))
        h8_pool = s_w8.enter_context(tc.tile_pool(name="h8", bufs=C3))
        h8_t = [h8_pool.tile([128, 2, T], F8, tag="h8", name=f"h8_{c}") for c in range(C3)]
        wqk_t, wv_t = [], []
        for c in range(C3):
            wt = wqk_pool.tile([128, 2, 2 * D], F8, tag="wqk", name=f"wqk_{c}")
            nc.gpsimd.dma_start(wt[:], wqk_d[c])
            wqk_t.append(wt)
            vt = wv_pool.tile([128, 2, VROW], F8, tag="wv", name=f"wvt_{c}")
            nc.gpsimd.dma_start(vt[:], wv_d[c])
            wv_t.append(vt)
        for c in range(4):
            nc.gpsimd.dma_start(wo_t[c][:], wo_d[c])

        def layer_norm2(y_tiles, psum_pool, ln_pool, emit):
            """on-device feature-axis LN (partition reduction via ones-matmul)"""
            sq_tiles = []
            for k in range(KC):
                sq = ln_pool.tile([128, T], F16, tag="sq", bufs=2)
                nc.scalar.activation(sq[:], y_tiles[k][:], AF.Square)
                sq_tiles.append(sq)
            ps_s = psum_pool.tile([128, T], F32, tag="mm")
            for k in range(KC):
                for hf in range(2):
                    nc.tensor.matmul(ps_s[:, HLF[hf]], ones[:], y_tiles[k][:, HLF[hf]],
                                     start=(k == 0), stop=(k == KC - 1))
            ps_q = psum_pool.tile([128, T], F32, tag="mm")
            for k in range(KC):
                for hf in range(2):
                    nc.tensor.matmul(ps_q[:, HLF[hf]], ones[:], sq_tiles[k][:, HLF[hf]],
                                     start=(k == 0), stop=(k == KC - 1))
            mu16 = ln_pool.tile([128, T], F16, tag="mu16", bufs=1)
            nc.vector.tensor_scalar_mul(mu16[:], ps_s[:], 1.0 / D)
            ms = ln_pool.tile([128, T], F16, tag="ms", bufs=1)
            nc.vector.tensor_scalar_mul(ms[:], ps_q[:], 1.0 / D)
            mu2 = ln_pool.tile([128, T], F16, tag="mu2", bufs=1)
            nc.scalar.activation(mu2[:], mu16[:], AF.Square)
            var = ln_pool.tile([128, T], F16, tag="var", bufs=1)
            nc.vector.tensor_tensor(var[:], ms[:], mu2[:], op=OP.subtract)
            lnv = ln_pool.tile([128, T], F16, tag="lnv", bufs=1)
            nc.scalar.activation(lnv[:], var[:], AF.Ln, bias=eps_sb[:])
            rs = ln_pool.tile([128, T], F16, tag="rs", bufs=1)
            nc.scalar.activation(rs[:], lnv[:], AF.Exp, scale=-0.5)
            for k in range(KC):
                t1 = ln_pool.tile([128, T], F16, tag="lnt1", bufs=2)
                nc.vector.tensor_tensor(t1[:], y_tiles[k][:], mu16[:], op=OP.subtract)
                t2 = ln_pool.tile([128, T], F16, tag="lnt2", bufs=2)
                nc.vector.tensor_tensor(t2[:], t1[:], rs[:], op=OP.mult)
                emit(k, t2)

        with tc.tile_pool(name="ps_a", bufs=2, space="PSUM") as ps_a:
            # ---- PE warm-up: repeated (idempotent) mask broadcast ----------
            mask_b = mb_pool.tile([128, T], F16, tag="maskb")
            for hf in range(2):
                pm = ps_a.tile([128, 512], F32, tag="b", bufs=2)
                for rep in range(8):
                    nc.tensor.matmul(pm[:], ones[0:1, :], mask_sb[:, HLF[hf]],
                                     start=True, stop=True)
                nc.vector.tensor_copy(mask_b[:, HLF[hf]], pm[:])
            # ---- LN1: host-computed row stats broadcast, normalize on DVE --
            with tc.tile_pool(name="ln1", bufs=1) as ln1_pool:
                mu16t = ln1_pool.tile([128, T], F16, tag="mu16", bufs=1)
                rs1t = ln1_pool.tile([128, T], F16, tag="rs", bufs=1)
                for i, dst in ((0, mu16t), (1, rs1t)):
                    for hf in range(2):
                        pr = ps_a.tile([128, 512], F32, tag="b", bufs=2)
                        nc.tensor.matmul(pr[:], ones[0:1, :], lnrow[:, i, HLF[hf]],
                                         start=True, stop=True)
                        nc.scalar.activation(dst[:, HLF[hf]], pr[:], AF.Copy)
                mu16 = mu16t[:]
                rs1 = rs1t[:]
                for hf in range(2):
                    pm2 = ps_a.tile([128, 512], F32, tag="b", bufs=2)
                    for rep in range(8):
                        nc.tensor.matmul(pm2[:], ones[0:1, :], mask_sb[:, HLF[hf]],
                                         start=True, stop=True)
                    nc.vector.tensor_copy(mask_b[:, HLF[hf]], pm2[:])
                for k in range(KC):
                    t1 = ln1_pool.tile([128, T], F16, tag="lnt1", bufs=2)
                    nc.vector.tensor_tensor(t1[:], x_tiles[k][:], mu16, op=OP.subtract)
                    t2 = ln1_pool.tile([128, T], F16, tag="lnt2", bufs=2)
                    nc.vector.tensor_tensor(t2[:], t1[:], rs1, op=OP.mult)
                    c, j = divmod(k, 2)
                    nc.scalar.activation(h8_t[c][:, j, :], t2[:], AF.Identity,
                                         scale=g1[:, k:k + 1], bias=b1[:, k:k + 1])

            # ---- q/k projection (fp8 DR, per-head M=96 blocks) -------------
            for b in range(16):
                dst = q_sb if b < 8 else k_sb
                scl = QS if b < 8 else 1.0
                hh = b % 8
                ps = ps_a.tile([96, T], F32, tag="qk", bufs=3)
                for hf in range(2):
                    for c in range(C3):
                        nc.tensor.matmul(
                            ps[:, HLF[hf]], wqk_t[c][:, :, 96 * b:96 * (b + 1)],
                            h8_t[c][:, :, HLF[hf]],
                            start=(c == 0), stop=(c == C3 - 1), perf_mode=DR)
                nc.scalar.activation(dst[:, hh * T: hh * T + T],
                                     ps[:], AF.Identity, scale=scl, bias=bqk[:, b:b + 1])

            # ---- v projection (token-major, fp8 DR) ------------------------
            for tt in range(G * NQT):
                for grp in range(2):
                    ps = ps_a.tile([128, VGRP], F32, tag="b", bufs=2)
                    for c in range(C3):
                        nc.tensor.matmul(
                            ps[:], h8_t[c][:, :, 128 * tt:128 * (tt + 1)],
                            wv_t[c][:, :, grp * VGRP:(grp + 1) * VGRP],
                            start=(c == 0), stop=(c == C3 - 1), perf_mode=DR)
                    nc.vector.tensor_tensor(
                        v_t[tt][:, grp * VGRP:(grp + 1) * VGRP], ps[:],
                        bvb[:, grp * VGRP:(grp + 1) * VGRP], op=OP.add)

        # ---- attention, software-pipelined: scores(i) | attn(i-1) | tail(i-2)
        # kt 0,1 add bias via identity-matmul on PE; kt 2,3 multiply host-exp'd
        # bias on DVE after the exp
        with tc.tile_pool(name="pb", bufs=14) as pb_pool, \
             tc.tile_pool(name="rsm", bufs=3) as rsm_pool, \
             tc.tile_pool(name="ps_sc", bufs=2, space="PSUM") as ps_sc, \
             tc.tile_pool(name="ps_at", bufs=4, space="PSUM") as ps_at:
            pf_dmas = [(wf1_t[i], wfc1_d[i]) for i in range(KC)]
            pf_i = 0
            NGH = G * H
            st_pb = [None] * NGH
            st_pa = [None] * NGH
            st_sum = [None] * NGH

            def emit_scores(gh):
                g, hh = divmod(gh, H)
                base = hh * T + g * N_NODE
                bt4 = eb_pool.tile([128, NQT, N_NODE], F16, tag="eb",
                                   name=f"bt4_{gh}")
                nc.sync.dma_start(bt4[:], bias_d[gh])
                pb_tiles = []
                for pair in range(2):
                    sc = ps_sc.tile([128, T], F32, tag="sc", name=f"sc_{gh}_{pair}")
                    for half in range(2):
                        kt = 2 * pair + half
                        nc.tensor.matmul(sc[:, HLF[half]],
                                         k_sb[:, base + kt * 128: base + (kt + 1) * 128],
                                         q_sb[:, base: base + N_NODE],
                                         start=True, stop=(pair == 1))
                        if pair == 0:
                            nc.tensor.matmul(sc[:, HLF[half]], ident[:], bt4[:, kt, :],
                                             start=False, stop=True)
                    pb = pb_pool.tile([128, T], F16, tag="pb", name=f"pb_{gh}_{pair}")
                    nc.scalar.activation(pb[:], sc[:], AF.Exp)
                    if pair == 1:
                        pbm = pb_pool.tile([128, T], F16, tag="pb", name=f"pbm_{gh}")
                        nc.gpsimd.tensor_tensor(
                            pbm[:], pb[:],
                            bt4[:, 2:4, :], op=OP.mult)
                        pb = pbm
                    pb_tiles.append(pb)
                st_pb[gh] = pb_tiles

            def emit_attn(gh):
                g, hh = divmod(gh, H)
                pa = ps_at.tile([97, N_NODE], F32, tag="at", name=f"pa_{gh}")
                for kt in range(NQT):
                    nc.tensor.matmul(
                        pa[:], v_t[g * NQT + kt][:, _vbase(hh):_vbase(hh) + 97],
                        st_pb[gh][kt // 2][:, HLF[kt % 2]],
                        start=(kt == 0), stop=(kt == NQT - 1))
                st_pa[gh] = pa
                sums_sb = rsm_pool.tile([1, N_NODE], F16, tag="r", name=f"sums_{gh}")
                nc.vector.tensor_copy(sums_sb[:], pa[96:97, :])
                st_sum[gh] = sums_sb

            def emit_tail(gh):
                g, hh = divmod(gh, H)
                pa = st_pa[gh]
                rb_ps = ps_at.tile([96, N_NODE], F32, tag="at", name=f"rbps_{gh}")
                nc.tensor.matmul(rb_ps[:], ones[0:1, 0:96], st_sum[gh][:],
                                 start=True, stop=True)
                rb32 = rsm_pool.tile([96, N_NODE], F32, tag="rb", name=f"rb32_{gh}")
                nc.vector.reciprocal_approx_fast(rb32[:], rb_ps[:])
                nc.vector.scalar_tensor_tensor(
                    attn8_t[hh // 2][0:96, hh % 2, g * 512:(g + 1) * 512],
                    pa[0:96, :], ASCL, rb32[:], op0=OP.mult, op1=OP.mult)

            for i in range(NGH + 2):
                if i < NGH:
                    emit_scores(i)
                if 1 <= i <= NGH:
                    emit_attn(i - 1)
                if i >= 2:
                    emit_tail(i - 2)
                if i % 3 == 2 and pf_i < len(pf_dmas):
                    t, dram = pf_dmas[pf_i]
                    nc.gpsimd.dma_start(t[:], dram)
                    pf_i += 1
            while pf_i < len(pf_dmas):
                t, dram = pf_dmas[pf_i]
                nc.gpsimd.dma_start(t[:], dram)
                pf_i += 1
        s_w8.close()
        s_eb.close()
        s_qv.close()

        # ---------------- phases 4+5, pipelined by graph half ----------------
        # out-proj(h0) -> {LN2(h0) || out-proj(h1)} -> {fc1(h0) || LN2(h1)} ...
        with tc.tile_pool(name="t4", bufs=4) as t4_pool, \
             tc.tile_pool(name="h2", bufs=KC) as h2_pool, \
             tc.tile_pool(name="wfc2", bufs=FC) as wfc2_pool, \
             tc.tile_pool(name="ln2", bufs=1) as ln2_pool, \
             tc.tile_pool(name="gelu", bufs=FC + 2) as gelu_pool, \
             tc.tile_pool(name="ps_c", bufs=6, space="PSUM") as ps_c:
            h2_tiles = [h2_pool.tile([128, T], F16, tag="h2", name=f"h2_{i}") for i in range(KC)]
            wf2_t = [wfc2_pool.tile([128, D], F16, tag="wfc2", name=f"wfc2_{i}") for i in range(FC)]
            for kk in range(FC):
                nc.gpsimd.dma_start(wf2_t[kk][:], wfc2_d[kk])

            y1_tiles = [y1_pool.tile([128, T], F16, tag="y1", name=f"y1_{m}") for m in range(KC)]

            def out_proj_half(hf):
                cs = HLF[hf]
                for m in range(KC):
                    po = ps_c.tile([128, 512], F32, tag="b", bufs=6)
                    for c in range(4):
                        nc.tensor.matmul(po[:], wo_t[c][:, :, 128 * m:128 * (m + 1)],
                                         attn8_t[c][:, :, cs],
                                         start=(c == 0), stop=(c == 3), perf_mode=DR)
                    t = t4_pool.tile([128, 512], F16, tag="tmp")
                    nc.vector.scalar_tensor_tensor(t[:], po[:], bout[:, m:m + 1],
                                                   mask_b[:, cs], op0=OP.add, op1=OP.mult)
                    nc.vector.tensor_tensor(y1_tiles[m][:, cs], t[:],
                                            x_tiles[m][:, cs], op=OP.add)

            def ln2_half(hf):
                cs = HLF[hf]
                sq_tiles = []
                for k in range(KC):
                    sq = ln2_pool.tile([128, 512], F16, tag="sq", bufs=3)
                    nc.scalar.activation(sq[:], y1_tiles[k][:, cs], AF.Square)
                    sq_tiles.append(sq)
                ps_s = ps_c.tile([128, 512], F32, tag="b", bufs=6)
                for k in range(KC):
                    nc.tensor.matmul(ps_s[:], ones[:], y1_tiles[k][:, cs],
                                     start=(k == 0), stop=(k == KC - 1))
                ps_q = ps_c.tile([128, 512], F32, tag="b", bufs=6)
                for k in range(KC):
                    nc.tensor.matmul(ps_q[:], ones[:], sq_tiles[k][:],
                                     start=(k == 0), stop=(k == KC - 1))
                mu16 = ln2_pool.tile([128, 512], F16, tag="mu16", bufs=2)
                nc.vector.tensor_scalar_mul(mu16[:], ps_s[:], 1.0 / D)
                ms = ln2_pool.tile([128, 512], F16, tag="ms", bufs=2)
                nc.vector.tensor_scalar_mul(ms[:], ps_q[:], 1.0 / D)
                mu2 = ln2_pool.tile([128, 512], F16, tag="mu2", bufs=2)
                nc.scalar.activation(mu2[:], mu16[:], AF.Square)
                var = ln2_pool.tile([128, 512], F16, tag="var", bufs=2)
                nc.vector.tensor_tensor(var[:], ms[:], mu2[:], op=OP.subtract)
                lnv = ln2_pool.tile([128, 512], F16, tag="lnv", bufs=2)
                nc.scalar.activation(lnv[:], var[:], AF.Ln, bias=eps_sb[:])
                rs = ln2_pool.tile([128, 512], F16, tag="rs", bufs=2)
                nc.scalar.activation(rs[:], lnv[:], AF.Exp, scale=-0.5)
                for k in range(KC):
                    t1 = ln2_pool.tile([128, 512], F16, tag="lnt1", bufs=2)
                    nc.vector.tensor_tensor(t1[:], y1_tiles[k][:, cs], mu16[:], op=OP.subtract)
                    t2 = ln2_pool.tile([128, 512], F16, tag="lnt2", bufs=2)
                    nc.vector.tensor_tensor(t2[:], t1[:], rs[:], op=OP.mult)
                    if k % 2 == 0:
                        nc.scalar.activation(h2_tiles[k][:, cs], t2[:], AF.Identity,
                                             scale=g2[:, k:k + 1], bias=b2[:, k:k + 1])
                    else:
                        nc.vector.tensor_scalar(h2_tiles[k][:, cs], t2[:],
                                                g2[:, k:k + 1], b2[:, k:k + 1],
                                                op0=OP.mult, op1=OP.add)

            gelu_tiles = [[None] * FC, [None] * FC]

            def fc1_half(hf):
                cs = HLF[hf]
                for n in range(FC):
                    pf = ps_c.tile([128, 512], F32, tag="b", bufs=6)
                    for k in range(KC):
                        nc.tensor.matmul(pf[:], wf1_t[k][:, n * 128:(n + 1) * 128],
                                         h2_tiles[k][:, cs],
                                         start=(k == 0), stop=(k == KC - 1))
                    gt = gelu_pool.tile([128, 512], F16, tag="gelu",
                                        name=f"gelu_{hf}_{n}")
                    nc.scalar.activation(gt[:], pf[:], AF.Gelu, bias=bfc1[:, n:n + 1])
                    gelu_tiles[hf][n] = gt

            def fc2_half(hf):
                cs = HLF[hf]
                for m in range(KC):
                    py = ps_c.tile([128, 512], F32, tag="b", bufs=6)
                    for kk in range(FC):
                        nc.tensor.matmul(py[:], wf2_t[kk][:, m * 128:(m + 1) * 128],
                                         gelu_tiles[hf][kk][:],
                                         start=(kk == 0), stop=(kk == FC - 1))
                    yo = yo_pool.tile([128, 512], F16, tag="yo")
                    nc.vector.scalar_tensor_tensor(yo[:], py[:], bfc2[:, m:m + 1],
                                                   y1_tiles[m][:, cs],
                                                   op0=OP.add, op1=OP.add)
                    nc.sync.dma_start(yt_d[m][:, cs], yo[:])

            out_proj_half(0)
            ln2_half(0)
            out_proj_half(1)
            ln2_half(1)
            fc1_half(0)
            fc2_half(0)
            fc1_half(1)
            fc2_half(1)

        s_attn.close()
        s_wout.close()
        s_x.close()

    nc.compile()
    return nc


def _get_runner():
    if "runner" in _cached:
        return _cached["runner"]
    import jax
    from jax.sharding import Mesh, PartitionSpec
    from jax.experimental.shard_map import shard_map
    import concourse.mybir as mybir
    from concourse.bass2jax import _bass_exec_p, install_neuronx_cc_hook, partition_id_tensor

    nc = _build()
    install_neuronx_cc_hook()
    partition_name = nc.partition_id_tensor.name if nc.partition_id_tensor else None
    in_names, out_names, out_avals, zero_outs = [], [], [], []
    for alloc in nc.m.functions[0].allocations:
        if not isinstance(alloc, mybir.MemoryLocationSet):
            continue
        name = alloc.memorylocations[0].name
        if alloc.kind == "ExternalInput":
            if name != partition_name:
                in_names.append(name)
        elif alloc.kind == "ExternalOutput":
            out_names.append(name)
            shape = tuple(alloc.tensor_shape)
            dtype = mybir.dt.np(alloc.dtype)
            out_avals.append(jax.core.ShapedArray(shape, dtype))
            zero_outs.append(np.zeros(shape, dtype))
    n_params = len(in_names)
    all_in_names = in_names + out_names + ([partition_name] if partition_name else [])

    def _body(*args):
        operands = list(args)
        if partition_name is not None:
            operands.append(partition_id_tensor())
        outs = _bass_exec_p.bind(
            *operands,
            out_avals=tuple(out_avals),
            in_names=tuple(all_in_names),
            out_names=tuple(out_names),
            lowering_input_output_aliases=(),
            sim_require_finite=False,
            sim_require_nnan=False,
            nc=nc,
        )
        return tuple(outs)

    donate = tuple(range(n_params, n_params + len(out_avals)))
    devices = jax.devices()[:NC]
    mesh = Mesh(np.asarray(devices), ("core",))
    in_specs = (PartitionSpec("core"),) * (n_params + len(out_avals))
    out_specs = (PartitionSpec("core"),) * len(out_names)
    sharded = jax.jit(
        shard_map(_body, mesh=mesh, in_specs=in_specs, out_specs=out_specs, check_rep=False),
        donate_argnums=donate, keep_unused=True,
    )

    runner = {
        "nc": nc, "sharded": sharded, "in_names": in_names,
        "out_names": out_names, "out_avals": out_avals, "zero_outs": zero_outs,
    }
    _cached["runner"] = runner
    return runner


def _q8(x):
    import ml_dtypes
    return np.clip(np.asarray(x, np.float32), -240, 240).astype(ml_dtypes.float8_e4m3)


def prep_inputs(x, attn_bias, node_non_padding_mask, in_w, in_b, out_w, out_b,
                ln1_g, ln1_b, fc1_w, fc1_b, fc2_w, fc2_b, ln2_g, ln2_b):
    """Host-side sharding/layout prep. Returns per-core dicts keyed by dram
    parameter name."""
    f16, f32 = np.float16, np.float32
    x = np.asarray(x, f32)
    xt = x.transpose(2, 1, 0).reshape(D, N_GRAPH * N_NODE).astype(f16)  # [768, 8192]
    xt_pc = [np.ascontiguousarray(xt[:, c * T:(c + 1) * T]).reshape(KC, 128, T) for c in range(NC)]
    # bias transposed to [k, q], partition-major batch [gh, 128, 4, 512];
    # k-tiles 2,3 hold exp(bias) (multiplied in after the exp, on GpSimd)
    biasT = np.asarray(attn_bias, f32).transpose(0, 2, 1).reshape(
        N_GRAPH * H, NQT, 128, N_NODE).transpose(0, 2, 1, 3).copy()
    biasT[:, :, 2:4, :] = np.exp(biasT[:, :, 2:4, :])
    biasT = biasT.astype(f16)
    mask = np.asarray(node_non_padding_mask).astype(f16) * np.float16(1.0 / ASCL)

    in_w = np.asarray(in_w, f32)
    in_b = np.asarray(in_b, f32).copy()
    in_b[:D] *= QS
    # q/k weights: [768 in, 1536 out] -> DoubleRow chunks [3, 128, 2, 1536]
    wqkT = in_w[:2 * D].T
    wqk8 = _q8(wqkT.reshape(C3, 2, 128, 2 * D).transpose(0, 2, 1, 3))
    bqk = np.zeros((128, 16), f32)
    bqk[:96] = in_b[:2 * D].reshape(16, 96).T
    # v weights token-major with per-head 97-col blocks (96 feats + ones col)
    wvT = in_w[2 * D:].T                                   # [768 in, 768 out]
    bv = in_b[2 * D:]
    wv_full = np.zeros((D, VROW), f32)
    bvb = np.zeros((128, VROW), f32)
    for h in range(H):
        b0 = _vbase(h)
        wv_full[:, b0:b0 + 96] = wvT[:, 96 * h:96 * h + 96]
        bvb[:, b0:b0 + 96] = bv[96 * h:96 * h + 96]
        bvb[:, b0 + 96] = 1.0
    wv8 = _q8(wv_full.reshape(C3, 2, 128, VROW).transpose(0, 2, 1, 3))
    # out-proj weights in the head-padded (8x128) attn feature space
    woT = np.asarray(out_w, f32).T                         # [768 attn-f, 768 out]
    wop = np.zeros((1024, D), f32)
    for h in range(H):
        wop[128 * h:128 * h + 96] = woT[96 * h:96 * h + 96]
    wo8 = _q8(wop.reshape(4, 2, 128, D).transpose(0, 2, 1, 3))

    # host-side LN1 row stats (input statistics; broadcast on device)
    xf = x  # [N, G, D] f32
    mu = xf.mean(-1)                                       # [N, G]
    var = xf.var(-1)
    rsr = 1.0 / np.sqrt(var + EPS)
    muT = mu.T.reshape(N_GRAPH * N_NODE)                   # token order [g, n]
    rsT = rsr.T.reshape(N_GRAPH * N_NODE)
    cb32 = np.zeros((128, 76), f32)
    cb32[:, 0:16] = bqk
    cb32[:, 16:22] = (np.asarray(out_b, f32) * ASCL).reshape(KC, 128).T
    cb32[:, 22:46] = np.asarray(fc1_b, f32).reshape(FC, 128).T
    cb32[:, 46:52] = np.asarray(fc2_b, f32).reshape(KC, 128).T
    cb32[:, 52:58] = np.asarray(ln1_g, f32).reshape(KC, 128).T
    cb32[:, 58:64] = np.asarray(ln1_b, f32).reshape(KC, 128).T
    cb32[:, 64:70] = np.asarray(ln2_g, f32).reshape(KC, 128).T
    cb32[:, 70:76] = np.asarray(ln2_b, f32).reshape(KC, 128).T
    cb16 = np.zeros((128, 1040), f16)
    cb16[:, 0:128] = np.ones((128, 128), f16)
    cb16[:, 128:256] = np.eye(128, dtype=f16)
    cb16[:, 256:256 + VROW] = bvb.astype(f16)
    shared = {
        "wqk8": np.ascontiguousarray(wqk8),
        "wv8": np.ascontiguousarray(wv8),
        "wo8": np.ascontiguousarray(wo8),
        "wfc1": np.ascontiguousarray(np.asarray(fc1_w, f32).T.astype(f16)).reshape(KC, 128, FFN),
        "wfc2": np.ascontiguousarray(np.asarray(fc2_w, f32).T.astype(f16)).reshape(FC, 128, D),
        "cb32": cb32,
        "cb16": cb16,
    }
    per_core = []
    for c in range(NC):
        m = dict(shared)
        m["xt"] = xt_pc[c]
        lr = np.zeros((1, 2, T), np.float16)
        lr[0, 0] = muT[c * T:(c + 1) * T]
        lr[0, 1] = rsT[c * T:(c + 1) * T]
        m["lnrow"] = lr
        m["biasT"] = np.ascontiguousarray(biasT[G * H * c: G * H * (c + 1)])
        m["maskrow"] = np.ascontiguousarray(mask[G * c: G * (c + 1)]).reshape(1, T)
        per_core.append(m)
    return per_core


def postprocess(outs):
    """outs: list of 8 per-core dicts with 'yt' [KC, 128, T] f16 -> [512, 16, 768]"""
    yt = np.stack([o["yt"].astype(np.float32).reshape(D, T) for o in outs])
    y = yt.reshape(NC, D, G, N_NODE).transpose(3, 0, 2, 1).reshape(N_NODE, N_GRAPH, D)
    return np.ascontiguousarray(y)


def run_per_core(per_core):
    r = _get_runner()
    n = NC
    concat_in = [
        np.concatenate([np.asarray(per_core[c][name]) for c in range(n)], axis=0)
        for name in r["in_names"]
    ]
    concat_zeros = [np.zeros((n * z.shape[0], *z.shape[1:]), z.dtype) for z in r["zero_outs"]]
    out_arrs = r["sharded"](*concat_in, *concat_zeros)
    return [
        {name: np.asarray(out_arrs[i]).reshape(n, *r["out_avals"][i].shape)[c]
         for i, name in enumerate(r["out_names"])}
        for c in range(n)
    ]


def kernel(**inputs):
    per_core = prep_inputs(**inputs)
    outs = run_per_core(per_core)
    return postprocess(outs)
